# revision 8
# baseline (speedup 1.0000x reference)
"""MixerLayerKAN Trainium2 kernel.

Computes, for x (B,T,C)=(32,512,512), fp32:
  token mixing : y = LN(x); per batch: F = fourier_feats(y^T) @ Wtok; z = (F...)
  channel mixing, each a FourierKAN (G=3) followed by a Linear, with residuals.

Strategy (data-parallel over batch, 4 batches per NeuronCore, weights
replicated, no collectives):

* Fourier features cos(kx), sin(kx) for k=1..3 are re-expressed in the
  Chebyshev product basis {s, c, s*c, s^2, s^3, c*s^2} with s=sin(x),
  c=cos(x).  The G-harmonic KAN coefficients fold (on host) into 6
  effective weight matrices, so the device only evaluates two ACT Sin
  ops per tile (sin(r), sin(r/2) after range-wrap; cos = 1-2*sin^2(r/2))
  plus 4 vector products.
* The constant term of cos(2x)=1-2s^2 and all KAN/linear biases fold into
  a single per-stage bias vector (host-side).
* All matmuls run in bf16 (fp32 PSUM accumulation): fp32 matmul is 4x
  slower on the PE.
* Layouts are chosen so only the second LN output needs a transpose
  (PE-transpose via identity).  The channel-mix linear uses Y2 as the
  stationary operand, which lands the result directly in (t, c) layout
  for the final residual.
* LN inv-std via bit-trick + 2 Newton iterations on the vector engine
  (avoids ACT table switches between Sin and Rsqrt sets).
"""

import numpy as np
import ml_dtypes

import concourse.bass as bass
import concourse.mybir as mybir
from concourse import tile
from concourse.vector_clock import ScopedClock
from concourse.bass_utils import run_bass_kernel_spmd
from concourse.masks import make_identity

AF = mybir.ActivationFunctionType
OP = mybir.AluOpType
FP32 = mybir.dt.float32
BF16 = mybir.dt.bfloat16
I32 = mybir.dt.int32

B, T, C, TD, G = 32, 512, 512, 256, 3
NCORES = 8
NB = B // NCORES          # batches per core
P = 128
EPS = 1e-5
PI = float(np.pi)
TWO_PI = float(2 * np.pi)
INV_2PI = float(1.0 / (2 * np.pi))
NF = 6                    # chebyshev basis features
NT = T // P               # 4 t-tiles
NC_ = C // P              # 4 c-tiles
NO_TOK = TD // P          # 2 token KAN output tiles
NO_CH = (2 * C) // P      # 8 channel KAN output tiles


class _TC(tile.TileContext):
    pass


def _split_multi_waits(nc):
    """This walrus build accepts at most ONE sync-wait command per
    instruction.  Tile emits several.  Fix: before each multi-wait
    instruction, splice in same-engine NOPs carrying one wait each (a wait
    executed earlier on the same engine is semantically identical)."""
    f = nc.m.functions[0]
    need = 0
    per_engine = {}
    for bb in f.blocks:
        for inst in bb.instructions:
            si = getattr(inst, "sync_info", None)
            if si is not None and si.on_wait and len(si.on_wait) > 1:
                need += len(si.on_wait) - 1
                per_engine[inst.engine] = per_engine.get(inst.engine, 0) + (
                    len(si.on_wait) - 1)
    if not need:
        return
    # Pre-create NOPs per engine (they append to the current bb; pop them out).
    nop_pool = {}
    for eng, cnt in per_engine.items():
        handles = []
        for _ in range(cnt):
            bi = nc.engines[eng].nop(nofuse=True)
            handles.append(bi.ins)
        nop_pool[eng] = handles
    created = {id(i) for h in nop_pool.values() for i in h}
    for bb in f.blocks:
        bb.instructions[:] = [i for i in bb.instructions if id(i) not in created]
    for bb in f.blocks:
        out = []
        for inst in bb.instructions:
            si = getattr(inst, "sync_info", None)
            if si is not None and si.on_wait and len(si.on_wait) > 1:
                waits = list(si.on_wait)
                si.on_wait = [waits[-1]]
                for w in waits[:-1]:
                    nop = nop_pool[inst.engine].pop()
                    nop.sync_info = mybir.SyncInfo(on_wait=[w], on_update=[])
                    out.append(nop)
            out.append(inst)
        bb.instructions[:] = out


def _cheb_weights(coef):
    """coef (2, O, I, G) -> effective basis weights (I, 6, O) for the
    {s, c, s*c, s^2, s^3, c*s^2} basis, plus the constant term (O,).

    cos(1x)=c; cos(2x)=1-2s^2; cos(3x)=c-4c s^2
    sin(1x)=s; sin(2x)=2 s c ; sin(3x)=3s-4s^3
    """
    cosw = coef[0]  # (O, I, G)
    sinw = coef[1]
    O, I, _ = cosw.shape
    w = np.zeros((I, NF, O), np.float64)
    w[:, 0, :] = (sinw[:, :, 0] + 3.0 * sinw[:, :, 2]).T      # s
    w[:, 1, :] = (cosw[:, :, 0] + cosw[:, :, 2]).T            # c
    w[:, 2, :] = (2.0 * sinw[:, :, 1]).T                      # s*c
    w[:, 3, :] = (-2.0 * cosw[:, :, 1]).T                     # s^2
    w[:, 4, :] = (-4.0 * sinw[:, :, 2]).T                     # s^3
    w[:, 5, :] = (-4.0 * cosw[:, :, 2]).T                     # c*s^2
    const = cosw[:, :, 1].sum(axis=1)                         # (O,) from the "1" of cos(2x)
    return w, const


def _build(apply_ln1, apply_ln2):
    nc = bass.Bass()
    x_in = nc.dram_tensor("x", [NB, T, C], FP32, kind="ExternalInput")
    y_out = nc.dram_tensor("y", [NB, T, C], FP32, kind="ExternalOutput")
    wtok_in = nc.dram_tensor("wtok", [NT, P, NF * TD], BF16, kind="ExternalInput")
    wch_in = nc.dram_tensor("wch", [NC_, P, NF * 2 * C], BF16, kind="ExternalInput")
    tlw_in = nc.dram_tensor("tlw", [NO_TOK, P, T], BF16, kind="ExternalInput")
    clw_in = nc.dram_tensor("clw", [NO_CH, P, C], BF16, kind="ExternalInput")
    btok_in = nc.dram_tensor("btok", [P, NT], FP32, kind="ExternalInput")
    bch_in = nc.dram_tensor("bch", [P, C], FP32, kind="ExternalInput")
    ln_in = nc.dram_tensor("lnwb", [P, 4 * C], FP32, kind="ExternalInput")

    with _TC(nc) as tc, \
         tc.tile_pool(name="singles", bufs=1) as singles, \
         tc.tile_pool(name="xpool", bufs=1) as xpool, \
         tc.tile_pool(name="xnpool", bufs=2) as xnpool, \
         tc.tile_pool(name="fpool", bufs=2) as fpool, \
         tc.tile_pool(name="ypool", bufs=2) as ypool, \
         tc.tile_pool(name="x1pool", bufs=1) as x1pool, \
         tc.tile_pool(name="opool", bufs=2) as opool, \
         tc.tile_pool(name="stats", bufs=2) as stats, \
         tc.tile_pool(name="psum", bufs=8, space="PSUM") as psum:

        # ---- resident weights/constants ----
        ident = singles.tile([P, P], FP32, tag="ident")
        make_identity(nc, ident)

        wtok = []
        for i in range(NT):
            t_ = singles.tile([P, NF, TD], BF16, tag=f"wtok{i}")
            nc.sync.dma_start(out=t_, in_=wtok_in[i].rearrange("p (f o) -> p f o", f=NF))
            wtok.append(t_)
        wch = []
        for m in range(NC_):
            t_ = singles.tile([P, NF, 2 * C], BF16, tag=f"wch{m}")
            nc.sync.dma_start(out=t_, in_=wch_in[m].rearrange("p (f o) -> p f o", f=NF))
            wch.append(t_)
        tlw = []
        for j in range(NO_TOK):
            t_ = singles.tile([P, T], BF16, tag=f"tlw{j}")
            nc.sync.dma_start(out=t_, in_=tlw_in[j])
            tlw.append(t_)
        clw = []
        for j in range(NO_CH):
            t_ = singles.tile([P, C], BF16, tag=f"clw{j}")
            nc.sync.dma_start(out=t_, in_=clw_in[j])
            clw.append(t_)
        btok = singles.tile([P, NT], FP32, tag="btok")
        nc.sync.dma_start(out=btok, in_=btok_in[:, :])
        bch = singles.tile([P, C], FP32, tag="bch")
        nc.sync.dma_start(out=bch, in_=bch_in[:, :])
        lnwb = singles.tile([P, 4, C], FP32, tag="lnwb")
        if apply_ln1 or apply_ln2:
            nc.sync.dma_start(out=lnwb, in_=ln_in.rearrange("p (k c) -> p k c", k=4))

        # ---- helpers ----
        def layernorm(src_tiles, dst_tiles, apply_wb, wb_idx):
            """LN over the free dim (c) of 4 (128, C) tiles."""
            mvs = stats.tile([P, NT, 2], FP32, tag="mvs", name="mvs")
            st6 = stats.tile([P, 6], FP32, tag="st6", name="st6")
            for i in range(NT):
                nc.vector.bn_stats(out=st6, in_=src_tiles[i])
                nc.vector.bn_aggr(out=mvs[:, i, :], in_=st6)
            var = mvs[:, :, 1]
            h = stats.tile([P, NT], FP32, tag="h", name="h")
            nc.vector.tensor_scalar(out=h, in0=var, scalar1=EPS, scalar2=-0.5,
                                    op0=OP.add, op1=OP.mult)
            yi = stats.tile([P, NT], I32, tag="yi", name="yi")
            nc.vector.tensor_scalar(out=yi, in0=var.bitcast(I32), scalar1=1,
                                    scalar2=None, op0=OP.logical_shift_right)
            nc.vector.tensor_scalar(out=yi, in0=yi, scalar1=-1, scalar2=0x5F3759DF,
                                    op0=OP.mult, op1=OP.add)
            rstd = yi.bitcast(FP32)
            a = stats.tile([P, NT], FP32, tag="a", name="a")
            for _ in range(2):
                nc.vector.tensor_mul(out=a, in0=rstd, in1=rstd)
                nc.vector.tensor_mul(out=a, in0=a, in1=h)
                nc.vector.scalar_tensor_tensor(out=rstd, in0=a, scalar=1.5, in1=rstd,
                                               op0=OP.add, op1=OP.mult)
            for i in range(NT):
                nc.vector.tensor_scalar(out=dst_tiles[i], in0=src_tiles[i],
                                        scalar1=mvs[:, i, 0:1], scalar2=rstd[:, i:i + 1],
                                        op0=OP.subtract, op1=OP.mult)
                if apply_wb:
                    nc.vector.tensor_mul(out=dst_tiles[i], in0=dst_tiles[i],
                                         in1=lnwb[:, wb_idx, :])
                    nc.vector.tensor_add(out=dst_tiles[i], in0=dst_tiles[i],
                                         in1=lnwb[:, wb_idx + 1, :])

        def features(xn, tags, fbufs):
            """From xn (128,512) fp32 make 6 bf16 basis tiles {s,c,sc,ss,sss,css}.

            tags: prefix for the 6 output-tile tags (ephemerals share tags).
            """
            ni = fpool.tile([P, 512], I32, tag="ni", name="ni", bufs=1)
            nc.vector.tensor_scalar_mul(out=ni, in0=xn, scalar1=INV_2PI)
            nf_ = fpool.tile([P, 512], FP32, tag="nf", name="nf", bufs=1)
            nc.vector.tensor_copy(out=nf_, in_=ni)
            r = fpool.tile([P, 512], FP32, tag="r", name="r")
            nc.vector.scalar_tensor_tensor(out=r, in0=nf_, scalar=-TWO_PI, in1=xn,
                                           op0=OP.mult, op1=OP.add)
            s = fpool.tile([P, 512], BF16, tag=f"{tags}s", name="s", bufs=fbufs)
            nc.scalar.activation(out=s, in_=r, func=AF.Sin)
            sh = fpool.tile([P, 512], FP32, tag="sh", name="sh")
            nc.scalar.activation(out=sh, in_=r, func=AF.Sin, scale=0.5)
            t2 = fpool.tile([P, 512], FP32, tag="t2", name="t2")
            nc.vector.tensor_mul(out=t2, in0=sh, in1=sh)
            c = fpool.tile([P, 512], BF16, tag=f"{tags}c", name="c", bufs=fbufs)
            nc.vector.tensor_scalar(out=c, in0=t2, scalar1=-2.0, scalar2=1.0,
                                    op0=OP.mult, op1=OP.add)
            ss = fpool.tile([P, 512], BF16, tag=f"{tags}ss", name="ss", bufs=fbufs)
            nc.vector.tensor_mul(out=ss, in0=s, in1=s)
            sc = fpool.tile([P, 512], BF16, tag=f"{tags}sc", name="sc", bufs=fbufs)
            nc.vector.tensor_mul(out=sc, in0=s, in1=c)
            sss = fpool.tile([P, 512], BF16, tag=f"{tags}sss", name="sss", bufs=fbufs)
            nc.vector.tensor_mul(out=sss, in0=ss, in1=s)
            css = fpool.tile([P, 512], BF16, tag=f"{tags}css", name="css", bufs=fbufs)
            nc.vector.tensor_mul(out=css, in0=ss, in1=c)
            return [s, c, sc, ss, sss, css]

        # ---- main pipeline over this core's batches ----
        for b in range(NB):
            # load x[b] as 4 (128, C) tiles
            X = []
            for i in range(NT):
                t_ = xpool.tile([P, C], FP32, tag=f"X{i}", name=f"X{i}")
                nc.sync.dma_start(out=t_, in_=x_in[b, i * P:(i + 1) * P, :])
                X.append(t_)

            # LN1 -> xn1 (t,c); xn1 tiles are consumed per-i right away
            xn1 = [xnpool.tile([P, C], FP32, tag="xn1", name="xn1", bufs=4)
                   for i in range(NT)]
            layernorm(X, xn1, apply_ln1, 0)

            # mm1 (i-streamed): Ytok[j] (o=128, c=512) held across the i loop
            ptok = [psum.tile([P, C], FP32, tag="ps", name="ptok", bufs=8)
                    for j in range(NO_TOK)]
            for i in range(NT):
                ftok = features(xn1[i], "t", 2)
                for j in range(NO_TOK):
                    for f in range(NF):
                        nc.tensor.matmul(ptok[j], wtok[i][:, f, j * P:(j + 1) * P],
                                         ftok[f], start=(i == 0 and f == 0),
                                         stop=(i == NT - 1 and f == NF - 1))
            ytok_sb = []
            for j in range(NO_TOK):
                ysb = ypool.tile([P, C], BF16, tag=f"ytok{j}", name="ysb")
                nc.scalar.copy(out=ysb, in_=ptok[j])
                ytok_sb.append(ysb)

            # mm2 + residual: x1[q] = X[q] + (tlw[j][:,q]T @ ytok[j] + btok[q])
            x1 = []
            for q in range(NT):
                pz = psum.tile([P, C], FP32, tag="ps", name="pz", bufs=8)
                for j in range(NO_TOK):
                    nc.tensor.matmul(pz, tlw[j][:, q * P:(q + 1) * P], ytok_sb[j],
                                     start=(j == 0), stop=(j == NO_TOK - 1))
                xt = x1pool.tile([P, C], FP32, tag=f"x1_{q}", name=f"x1_{q}")
                nc.vector.scalar_tensor_tensor(out=xt, in0=pz, scalar=btok[:, q:q + 1],
                                               in1=X[q], op0=OP.add, op1=OP.add)
                x1.append(xt)

            # LN2 -> xn2 (t,c)
            xn2 = [xnpool.tile([P, C], FP32, tag="xn2", name="xn2", bufs=4)
                   for i in range(NT)]
            layernorm(x1, xn2, apply_ln2, 2)

            # transpose xn2 -> xn2T (c,t)
            xn2T = [xnpool.tile([P, T], FP32, tag=f"xn2T_{m}", name=f"xn2T_{m}",
                                bufs=1) for m in range(NC_)]
            for i in range(NT):
                for m in range(NC_):
                    ptr = psum.tile([P, P], FP32, tag="ps", name="ptr", bufs=8,
                                    padded_shape=[P, C])
                    nc.tensor.transpose(ptr, xn2[i][:, m * P:(m + 1) * P], ident)
                    nc.vector.tensor_copy(out=xn2T[m][:, i * P:(i + 1) * P], in_=ptr)

            # channel features for all c-tiles (kept resident)
            fch = [features(xn2T[m], f"c{m}", 1) for m in range(NC_)]

            # mm3 in two j-groups of 4 banks:
            #   Y2[j] (o=128, t=512) += Wch[m][:, f, j]T @ fch[m][f]
            y2_sb = []
            for jg in range(2):
                pts = [psum.tile([P, T], FP32, tag="ps", name="py2", bufs=8)
                       for _ in range(4)]
                for m in range(NC_):
                    for jj, pt in enumerate(pts):
                        j = jg * 4 + jj
                        for f in range(NF):
                            nc.tensor.matmul(pt, wch[m][:, f, j * P:(j + 1) * P],
                                             fch[m][f], start=(m == 0 and f == 0),
                                             stop=(m == NC_ - 1 and f == NF - 1))
                for jj, pt in enumerate(pts):
                    j = jg * 4 + jj
                    ysb = ypool.tile([P, T], BF16, tag=f"y2_{j}", name="y2sb", bufs=1)
                    nc.scalar.copy(out=ysb, in_=pt)
                    y2_sb.append(ysb)

            # mm4 + bias + residual: out[q] (t=128, c=512)
            for q in range(NT):
                po = psum.tile([P, C], FP32, tag="ps", name="po", bufs=8)
                for j in range(NO_CH):
                    nc.tensor.matmul(po, y2_sb[j][:, q * P:(q + 1) * P], clw[j],
                                     start=(j == 0), stop=(j == NO_CH - 1))
                ot = opool.tile([P, C], FP32, tag="out", name="out")
                nc.vector.tensor_add(out=ot, in0=po, in1=bch)
                nc.vector.tensor_add(out=ot, in0=ot, in1=x1[q])
                nc.sync.dma_start(out=y_out[b, q * P:(q + 1) * P, :], in_=ot)

    _split_multi_waits(nc)
    return nc


_CACHE = {}


def _get_nc(apply_ln1, apply_ln2):
    key = (apply_ln1, apply_ln2)
    if key not in _CACHE:
        _CACHE[key] = _build(apply_ln1, apply_ln2)
    return _CACHE[key]


def prepare_in_maps(inputs):
    return _prepare(**inputs)


def _prepare(x, ln1_w, ln1_b, tok_coef, tok_kbias, tok_lw, tok_lb,
             ln2_w, ln2_b, ch_coef, ch_kbias, ch_lw, ch_lb):
    x = np.asarray(x, np.float32)
    f64 = np.float64

    # --- host-side weight folding ---
    wtok_eff, tok_const = _cheb_weights(np.asarray(tok_coef, f64))  # (T, 6, TD), (TD,)
    wch_eff, ch_const = _cheb_weights(np.asarray(ch_coef, f64))     # (C, 6, 2C), (2C,)

    kbias_tok = np.asarray(tok_kbias, f64).reshape(-1) + tok_const          # (TD,)
    kbias_ch = np.asarray(ch_kbias, f64).reshape(-1) + ch_const             # (2C,)
    bias_tok = np.asarray(tok_lb, f64) + np.asarray(tok_lw, f64) @ kbias_tok  # (T,)
    bias_ch = np.asarray(ch_lb, f64) + np.asarray(ch_lw, f64) @ kbias_ch      # (C,)

    wtok_np = wtok_eff.reshape(NT, P, NF * TD).astype(ml_dtypes.bfloat16)
    wch_np = wch_eff.reshape(NC_, P, NF * 2 * C).astype(ml_dtypes.bfloat16)
    tlw_np = np.ascontiguousarray(np.asarray(tok_lw, f64).T).reshape(
        NO_TOK, P, T).astype(ml_dtypes.bfloat16)
    clw_np = np.ascontiguousarray(np.asarray(ch_lw, f64).T).reshape(
        NO_CH, P, C).astype(ml_dtypes.bfloat16)
    btok_np = np.ascontiguousarray(
        bias_tok.reshape(NT, P).T).astype(np.float32)           # (P, NT)
    bch_np = np.broadcast_to(bias_ch.astype(np.float32), (P, C)).copy()
    lnwb_np = np.broadcast_to(
        np.concatenate([np.asarray(ln1_w, f64), np.asarray(ln1_b, f64),
                        np.asarray(ln2_w, f64), np.asarray(ln2_b, f64)]).astype(
            np.float32), (P, 4 * C)).copy()

    apply_ln1 = not (np.all(np.asarray(ln1_w) == 1.0) and np.all(np.asarray(ln1_b) == 0.0))
    apply_ln2 = not (np.all(np.asarray(ln2_w) == 1.0) and np.all(np.asarray(ln2_b) == 0.0))

    shared = dict(wtok=wtok_np, wch=wch_np, tlw=tlw_np, clw=clw_np,
                  btok=btok_np, bch=bch_np, lnwb=lnwb_np)
    in_maps = []
    for core in range(NCORES):
        m = dict(shared)
        m["x"] = np.ascontiguousarray(x[core * NB:(core + 1) * NB])
        in_maps.append(m)
    return {"build_key": (apply_ln1, apply_ln2), "in_maps": in_maps}


def kernel(**inputs):
    prep = _prepare(**inputs)
    nc = _get_nc(*prep["build_key"])
    res = run_bass_kernel_spmd(nc, prep["in_maps"], list(range(NCORES)))
    return np.concatenate([res.results[i]["y"] for i in range(NCORES)], axis=0)


# revision 9
# speedup vs baseline: 1.5850x; 1.5850x over previous
"""MixerLayerKAN Trainium2 kernel.

x (B,T,C)=(32,512,512) fp32; token-mix FourierKAN(T->TD)+Linear, then
channel-mix FourierKAN(C->2C)+Linear, LN + residual around each.

Strategy (data-parallel over batch, 4 batches per NeuronCore, weights
replicated, no collectives):

* Fourier features cos(kx)/sin(kx), k=1..3, re-expressed in the product
  basis {s, c, s*c, s^2, s^3, c*s^2} (s=sin x, c=cos x); the harmonic
  coefficients fold host-side into 6 effective weight matrices.  On
  device: range-wrap, two ACT Sin ops (sin r, sin r/2), cos via
  1-2sin^2(r/2), 4 vector products.
* Channel KAN (hidden 1024 > out 512): the post-KAN Linear folds into
  the KAN weights host-side (W @ lwT, fp64) -- 96 instead of 224
  matmuls per batch.  Token KAN keeps its 256-wide bottleneck 2-stage.
* All KAN/linear biases and the cos(2x) constant fold into one bias
  vector per mixing stage (host-side).
* All matmuls bf16 (fp32 PSUM accumulation).
* LN inv-std via bit-trick + 2 Newton iterations on DVE (avoids ACT
  table switches between the Sin and Sqrt table sets).
* Two-stage software pipeline across batches: batch b+1's LN/feature
  (DVE/ACT) work is emitted interleaved with batch b's channel-mix
  matmuls so the PE never starves at batch boundaries.
"""

import numpy as np
import ml_dtypes

import concourse.bass as bass
import concourse.mybir as mybir
from concourse import tile
from concourse.bass_utils import run_bass_kernel_spmd
from concourse.masks import make_identity

AF = mybir.ActivationFunctionType
OP = mybir.AluOpType
FP32 = mybir.dt.float32
BF16 = mybir.dt.bfloat16
I32 = mybir.dt.int32

B, T, C, TD, G = 32, 512, 512, 256, 3
NCORES = 8
NB = B // NCORES          # batches per core
P = 128
EPS = 1e-5
TWO_PI = float(2 * np.pi)
INV_2PI = float(1.0 / (2 * np.pi))
NF = 6                    # chebyshev basis features
NT = T // P               # 4 t-tiles
NC_ = C // P              # 4 c-tiles
NO_TOK = TD // P          # 2 token KAN hidden tiles


def _split_multi_waits(nc):
    """This walrus build accepts at most ONE sync-wait command per
    instruction.  Tile emits several.  Fix: before each multi-wait
    instruction, splice in same-engine NOPs carrying one wait each (a wait
    executed earlier on the same engine is semantically identical)."""
    f = nc.m.functions[0]
    per_engine = {}
    for bb in f.blocks:
        for inst in bb.instructions:
            si = getattr(inst, "sync_info", None)
            if si is not None and si.on_wait and len(si.on_wait) > 1:
                per_engine[inst.engine] = per_engine.get(inst.engine, 0) + (
                    len(si.on_wait) - 1)
    if not per_engine:
        return
    nop_pool = {}
    for eng, cnt in per_engine.items():
        nop_pool[eng] = [nc.engines[eng].nop(nofuse=True).ins for _ in range(cnt)]
    created = {id(i) for h in nop_pool.values() for i in h}
    for bb in f.blocks:
        bb.instructions[:] = [i for i in bb.instructions if id(i) not in created]
    for bb in f.blocks:
        out = []
        for inst in bb.instructions:
            si = getattr(inst, "sync_info", None)
            if si is not None and si.on_wait and len(si.on_wait) > 1:
                waits = list(si.on_wait)
                si.on_wait = [waits[-1]]
                for w in waits[:-1]:
                    nop = nop_pool[inst.engine].pop()
                    nop.sync_info = mybir.SyncInfo(on_wait=[w], on_update=[])
                    out.append(nop)
            out.append(inst)
        bb.instructions[:] = out


def _cheb_weights(coef):
    """coef (2, O, I, G) -> effective basis weights (I, 6, O) for the
    {s, c, s*c, s^2, s^3, c*s^2} basis, plus the constant term (O,).

    cos(1x)=c; cos(2x)=1-2s^2; cos(3x)=c-4c s^2
    sin(1x)=s; sin(2x)=2 s c ; sin(3x)=3s-4s^3
    """
    cosw = coef[0]  # (O, I, G)
    sinw = coef[1]
    O, I, _ = cosw.shape
    w = np.zeros((I, NF, O), np.float64)
    w[:, 0, :] = (sinw[:, :, 0] + 3.0 * sinw[:, :, 2]).T      # s
    w[:, 1, :] = (cosw[:, :, 0] + cosw[:, :, 2]).T            # c
    w[:, 2, :] = (2.0 * sinw[:, :, 1]).T                      # s*c
    w[:, 3, :] = (-2.0 * cosw[:, :, 1]).T                     # s^2
    w[:, 4, :] = (-4.0 * sinw[:, :, 2]).T                     # s^3
    w[:, 5, :] = (-4.0 * cosw[:, :, 2]).T                     # c*s^2
    const = cosw[:, :, 1].sum(axis=1)                         # from the "1" of cos(2x)
    return w, const


def _build(apply_ln1, apply_ln2):
    nc = bass.Bass()
    x_in = nc.dram_tensor("x", [NB, T, C], FP32, kind="ExternalInput")
    y_out = nc.dram_tensor("y", [NB, T, C], FP32, kind="ExternalOutput")
    wtok_in = nc.dram_tensor("wtok", [NT, P, NF * TD], BF16, kind="ExternalInput")
    wchf_in = nc.dram_tensor("wchf", [NC_, P, NF * C], BF16, kind="ExternalInput")
    tlw_in = nc.dram_tensor("tlw", [NO_TOK, P, T], BF16, kind="ExternalInput")
    btok_in = nc.dram_tensor("btok", [P, NT], FP32, kind="ExternalInput")
    bch_in = nc.dram_tensor("bch", [P, C], FP32, kind="ExternalInput")
    ln_in = nc.dram_tensor("lnwb", [P, 4 * C], FP32, kind="ExternalInput")

    with tile.TileContext(nc) as tc, \
         tc.tile_pool(name="singles", bufs=1) as singles, \
         tc.tile_pool(name="xpool", bufs=2) as xpool, \
         tc.tile_pool(name="xnpool", bufs=2) as xnpool, \
         tc.tile_pool(name="fpool", bufs=2) as fpool, \
         tc.tile_pool(name="ypool", bufs=2) as ypool, \
         tc.tile_pool(name="x1pool", bufs=2) as x1pool, \
         tc.tile_pool(name="opool", bufs=3) as opool, \
         tc.tile_pool(name="stats", bufs=2) as stats, \
         tc.tile_pool(name="psum", bufs=8, space="PSUM") as psum:

        # ---- batch-0 activations first so the big weight DMAs don't
        #      block the pipeline start ----
        def load_x(b):
            X = []
            for i in range(NT):
                t_ = xpool.tile([P, C], FP32, tag=f"X{i}", name=f"X{i}")
                nc.sync.dma_start(out=t_, in_=x_in[b, i * P:(i + 1) * P, :])
                X.append(t_)
            return X

        X0 = load_x(0)

        ident = singles.tile([P, P], FP32, tag="ident")
        make_identity(nc, ident)
        wtok = []
        for i in range(NT):
            t_ = singles.tile([P, NF, TD], BF16, tag=f"wtok{i}")
            nc.sync.dma_start(out=t_, in_=wtok_in[i].rearrange("p (f o) -> p f o", f=NF))
            wtok.append(t_)
        tlw = []
        for j in range(NO_TOK):
            t_ = singles.tile([P, T], BF16, tag=f"tlw{j}")
            nc.sync.dma_start(out=t_, in_=tlw_in[j])
            tlw.append(t_)
        btok = singles.tile([P, NT], FP32, tag="btok")
        nc.sync.dma_start(out=btok, in_=btok_in[:, :])
        wchf = []
        for m in range(NC_):
            t_ = singles.tile([P, NF, C], BF16, tag=f"wchf{m}")
            nc.sync.dma_start(out=t_, in_=wchf_in[m].rearrange("p (f o) -> p f o", f=NF))
            wchf.append(t_)
        bch = singles.tile([P, C], FP32, tag="bch")
        nc.sync.dma_start(out=bch, in_=bch_in[:, :])
        lnwb = singles.tile([P, 4, C], FP32, tag="lnwb")
        if apply_ln1 or apply_ln2:
            nc.sync.dma_start(out=lnwb, in_=ln_in.rearrange("p (k c) -> p k c", k=4))

        # ---- helpers ----
        def layernorm(src_tiles, dst_tiles, apply_wb, wb_idx):
            """LN over the free dim (c) of 4 (128, C) tiles."""
            mvs = stats.tile([P, NT, 2], FP32, tag="mvs", name="mvs")
            st6 = stats.tile([P, 6], FP32, tag="st6", name="st6")
            for i in range(NT):
                nc.vector.bn_stats(out=st6, in_=src_tiles[i])
                nc.vector.bn_aggr(out=mvs[:, i, :], in_=st6)
            var = mvs[:, :, 1]
            h = stats.tile([P, NT], FP32, tag="h", name="h")
            nc.vector.tensor_scalar(out=h, in0=var, scalar1=EPS, scalar2=-0.5,
                                    op0=OP.add, op1=OP.mult)
            yi = stats.tile([P, NT], I32, tag="yi", name="yi")
            nc.vector.tensor_scalar(out=yi, in0=var.bitcast(I32), scalar1=1,
                                    scalar2=None, op0=OP.logical_shift_right)
            nc.vector.tensor_scalar(out=yi, in0=yi, scalar1=-1, scalar2=0x5F3759DF,
                                    op0=OP.mult, op1=OP.add)
            rstd = yi.bitcast(FP32)
            a = stats.tile([P, NT], FP32, tag="a", name="a")
            for _ in range(2):
                nc.vector.tensor_mul(out=a, in0=rstd, in1=rstd)
                nc.vector.tensor_mul(out=a, in0=a, in1=h)
                nc.vector.scalar_tensor_tensor(out=rstd, in0=a, scalar=1.5, in1=rstd,
                                               op0=OP.add, op1=OP.mult)
            for i in range(NT):
                nc.vector.tensor_scalar(out=dst_tiles[i], in0=src_tiles[i],
                                        scalar1=mvs[:, i, 0:1], scalar2=rstd[:, i:i + 1],
                                        op0=OP.subtract, op1=OP.mult)
                if apply_wb:
                    nc.vector.tensor_mul(out=dst_tiles[i], in0=dst_tiles[i],
                                         in1=lnwb[:, wb_idx, :])
                    nc.vector.tensor_add(out=dst_tiles[i], in0=dst_tiles[i],
                                         in1=lnwb[:, wb_idx + 1, :])

        def features(xn, tags):
            """From xn (128,512) fp32 make 6 bf16 basis tiles {s,c,sc,ss,sss,css}."""
            ni = fpool.tile([P, 512], I32, tag="ni", name="ni")
            nc.vector.tensor_scalar_mul(out=ni, in0=xn, scalar1=INV_2PI)
            nf_ = fpool.tile([P, 512], FP32, tag="nf", name="nf")
            nc.vector.tensor_copy(out=nf_, in_=ni)
            r = fpool.tile([P, 512], FP32, tag="r", name="r")
            nc.vector.scalar_tensor_tensor(out=r, in0=nf_, scalar=-TWO_PI, in1=xn,
                                           op0=OP.mult, op1=OP.add)
            s = fpool.tile([P, 512], BF16, tag=f"{tags}s", name="s")
            nc.scalar.activation(out=s, in_=r, func=AF.Sin)
            sh = fpool.tile([P, 512], FP32, tag="sh", name="sh")
            nc.scalar.activation(out=sh, in_=r, func=AF.Sin, scale=0.5)
            t2 = fpool.tile([P, 512], FP32, tag="t2", name="t2")
            nc.vector.tensor_mul(out=t2, in0=sh, in1=sh)
            c = fpool.tile([P, 512], BF16, tag=f"{tags}c", name="c")
            nc.vector.tensor_scalar(out=c, in0=t2, scalar1=-2.0, scalar2=1.0,
                                    op0=OP.mult, op1=OP.add)
            ss = fpool.tile([P, 512], BF16, tag=f"{tags}ss", name="ss")
            nc.vector.tensor_mul(out=ss, in0=s, in1=s)
            sc = fpool.tile([P, 512], BF16, tag=f"{tags}sc", name="sc")
            nc.vector.tensor_mul(out=sc, in0=s, in1=c)
            sss = fpool.tile([P, 512], BF16, tag=f"{tags}sss", name="sss")
            nc.vector.tensor_mul(out=sss, in0=ss, in1=s)
            css = fpool.tile([P, 512], BF16, tag=f"{tags}css", name="css")
            nc.vector.tensor_mul(out=css, in0=ss, in1=c)
            return [s, c, sc, ss, sss, css]

        # ---- pipeline stages ----
        def stage1(b, X):
            """token mixing for batch b: LN1, token KAN (mm1), linear (mm2),
            residual, LN2.  Returns (x1, xn2)."""
            xn1 = [xnpool.tile([P, C], FP32, tag="xn1", name="xn1", bufs=5)
                   for _ in range(NT)]
            layernorm(X, xn1, apply_ln1, 0)
            ptok = [psum.tile([P, C], FP32, tag="ps", name="ptok", bufs=8)
                    for _ in range(NO_TOK)]
            for i in range(NT):
                ftok = features(xn1[i], "t")
                for j in range(NO_TOK):
                    for f in range(NF):
                        nc.tensor.matmul(ptok[j], wtok[i][:, f, j * P:(j + 1) * P],
                                         ftok[f], start=(i == 0 and f == 0),
                                         stop=(i == NT - 1 and f == NF - 1))
            ytok_sb = []
            for j in range(NO_TOK):
                ysb = ypool.tile([P, C], BF16, tag=f"ytok{j}", name="ysb")
                nc.scalar.copy(out=ysb, in_=ptok[j])
                ytok_sb.append(ysb)
            x1 = []
            for q in range(NT):
                pz = psum.tile([P, C], FP32, tag="ps", name="pz", bufs=8)
                for j in range(NO_TOK):
                    nc.tensor.matmul(pz, tlw[j][:, q * P:(q + 1) * P], ytok_sb[j],
                                     start=(j == 0), stop=(j == NO_TOK - 1))
                xt = x1pool.tile([P, C], FP32, tag=f"x1_{q}", name=f"x1_{q}")
                nc.vector.scalar_tensor_tensor(out=xt, in0=pz, scalar=btok[:, q:q + 1],
                                               in1=X[q], op0=OP.add, op1=OP.add)
                x1.append(xt)
            xn2 = [xnpool.tile([P, C], FP32, tag="xn2", name="xn2", bufs=5)
                   for _ in range(NT)]
            layernorm(x1, xn2, apply_ln2, 2)
            return x1, xn2

        def stage2_transpose(xn2):
            xn2T = [xnpool.tile([P, T], FP32, tag=f"xn2T_{m}", name=f"xn2T_{m}",
                                bufs=2) for m in range(NC_)]
            for i in range(NT):
                for m in range(NC_):
                    ptr = psum.tile([P, P], FP32, tag="ps", name="ptr", bufs=8,
                                    padded_shape=[P, C])
                    nc.tensor.transpose(ptr, xn2[i][:, m * P:(m + 1) * P], ident)
                    nc.vector.tensor_copy(out=xn2T[m][:, i * P:(i + 1) * P], in_=ptr)
            return xn2T

        def stage2_channel(b, x1, xn2T, emit_mid=None):
            """channel mixing (fused KAN+linear) + residual + store.

            emit_mid: optional callback invoked between c-tile iterations so
            the next batch's stage1 work can interleave in emission order."""
            pout = [psum.tile([P, C], FP32, tag="ps", name="pout", bufs=8)
                    for _ in range(NT)]
            for m in range(NC_):
                fc = features(xn2T[m], "c")
                for q in range(NT):
                    for f in range(NF):
                        nc.tensor.matmul(pout[q], fc[f][:, q * P:(q + 1) * P],
                                         wchf[m][:, f, :],
                                         start=(m == 0 and f == 0),
                                         stop=(m == NC_ - 1 and f == NF - 1))
                if emit_mid is not None:
                    emit_mid(m)
            for q in range(NT):
                ot = opool.tile([P, C], FP32, tag="out", name="out")
                nc.vector.tensor_add(out=ot, in0=pout[q], in1=bch)
                nc.vector.tensor_add(out=ot, in0=ot, in1=x1[q])
                nc.sync.dma_start(out=y_out[b, q * P:(q + 1) * P, :], in_=ot)

        # ---- software-pipelined emission over batches ----
        # stage1(b+1) instructions are emitted interleaved with batch b's
        # channel matmuls so DVE/ACT work overlaps the PE-heavy phase.
        state = {}
        X = X0
        x1, xn2 = stage1(0, X)
        state[0] = (x1, xn2)
        for b in range(NB):
            x1, xn2 = state.pop(b)
            xn2T = stage2_transpose(xn2)

            nxt = {}
            def emit_mid(m, b=b, nxt=nxt):
                if b + 1 >= NB:
                    return
                if m == 0:
                    nxt["X"] = load_x(b + 1)
                elif m == 2:
                    nxt["s1"] = stage1(b + 1, nxt["X"])

            stage2_channel(b, x1, xn2T, emit_mid)
            if b + 1 < NB:
                state[b + 1] = nxt["s1"]

    _split_multi_waits(nc)
    return nc


_CACHE = {}


def _get_nc(apply_ln1, apply_ln2):
    key = (apply_ln1, apply_ln2)
    if key not in _CACHE:
        _CACHE[key] = _build(apply_ln1, apply_ln2)
    return _CACHE[key]


def prepare_in_maps(inputs):
    return _prepare(**inputs)


def _prepare(x, ln1_w, ln1_b, tok_coef, tok_kbias, tok_lw, tok_lb,
             ln2_w, ln2_b, ch_coef, ch_kbias, ch_lw, ch_lb):
    x = np.asarray(x, np.float32)
    f64 = np.float64

    wtok_eff, tok_const = _cheb_weights(np.asarray(tok_coef, f64))  # (T,6,TD)
    wch_eff, ch_const = _cheb_weights(np.asarray(ch_coef, f64))     # (C,6,2C)

    kbias_tok = np.asarray(tok_kbias, f64).reshape(-1) + tok_const
    kbias_ch = np.asarray(ch_kbias, f64).reshape(-1) + ch_const
    bias_tok = np.asarray(tok_lb, f64) + np.asarray(tok_lw, f64) @ kbias_tok
    bias_ch = np.asarray(ch_lb, f64) + np.asarray(ch_lw, f64) @ kbias_ch

    # fold the channel post-KAN linear into the KAN weights (fp64)
    wchf = np.einsum("cfo,ko->cfk", wch_eff, np.asarray(ch_lw, f64))  # (C,6,C)

    wtok_np = wtok_eff.reshape(NT, P, NF * TD).astype(ml_dtypes.bfloat16)
    wchf_np = wchf.reshape(NC_, P, NF * C).astype(ml_dtypes.bfloat16)
    tlw_np = np.ascontiguousarray(np.asarray(tok_lw, f64).T).reshape(
        NO_TOK, P, T).astype(ml_dtypes.bfloat16)
    btok_np = np.ascontiguousarray(bias_tok.reshape(NT, P).T).astype(np.float32)
    bch_np = np.broadcast_to(bias_ch.astype(np.float32), (P, C)).copy()
    lnwb_np = np.broadcast_to(
        np.concatenate([np.asarray(ln1_w, f64), np.asarray(ln1_b, f64),
                        np.asarray(ln2_w, f64), np.asarray(ln2_b, f64)]).astype(
            np.float32), (P, 4 * C)).copy()

    apply_ln1 = not (np.all(np.asarray(ln1_w) == 1.0) and np.all(np.asarray(ln1_b) == 0.0))
    apply_ln2 = not (np.all(np.asarray(ln2_w) == 1.0) and np.all(np.asarray(ln2_b) == 0.0))

    shared = dict(wtok=wtok_np, wchf=wchf_np, tlw=tlw_np,
                  btok=btok_np, bch=bch_np, lnwb=lnwb_np)
    in_maps = []
    for core in range(NCORES):
        m = dict(shared)
        m["x"] = np.ascontiguousarray(x[core * NB:(core + 1) * NB])
        in_maps.append(m)
    return {"build_key": (apply_ln1, apply_ln2), "in_maps": in_maps}


def kernel(**inputs):
    prep = _prepare(**inputs)
    nc = _get_nc(*prep["build_key"])
    res = run_bass_kernel_spmd(nc, prep["in_maps"], list(range(NCORES)))
    return np.concatenate([res.results[i]["y"] for i in range(NCORES)], axis=0)
